# revision 1
# baseline (speedup 1.0000x reference)
"""BuddyPool kernel for Trainium2 (Bass/Tile), 8-core data-parallel.

Problem: cue (64,5,1024), patches (64,32,32,1024) ->
  sim = einsum('bkd,bhwd->bkhw'); idx = argmax(sim over hw);
  roi = mean of boundary-clamped 3x3 patch window around idx  -> (64,5,1024)

Sharding: batch across 8 cores, 8 samples/core. Inside one core:
  - load patches[s] (4 MiB) to SBUF in natural [hw, d] layout
  - PE-transpose 128x128 tiles to get patchesT [d, hw] (sim contracts d,
    and the PE contracts along the partition dim)
  - sim matmul: cueT[128,5].T @ patchesT[128,1024] accumulated over 8 d-chunks
  - argmax via DVE max/max_index (first-max tie rule == jnp.argmax)
  - index math on DVE -> 9 clamped window row ids + validity/count weights
  - indirect-DMA gather of 45 rows (5k x 9) from patches in DRAM
  - roi = W[45,5].T @ G[45,1024] with W = one-hot(k) * valid * (1/count)
"""

import sys

if "/opt/trn_rl_repo" not in sys.path:
    sys.path.insert(0, "/opt/trn_rl_repo")

import numpy as np

import concourse.bass as bass
import concourse.tile as tile
from concourse import mybir
from concourse.masks import make_identity

P = 128
B = 64          # full batch
NCORES = 8
NS = B // NCORES  # samples per core
K = 5
D = 1024
H = W = 32
HW = H * W      # 1024
NDC = D // P    # 8 d-chunks
NHWC = HW // P  # 8 hw-chunks
F32 = mybir.dt.float32
F32R = mybir.dt.float32r
U32 = mybir.dt.uint32

BF16 = mybir.dt.bfloat16

# Matmul dtype modes (hardware-validated via rel-err check):
#   SIM_DT:   dtype of patchesT/cueT tiles feeding the sim matmul.
#             F32 = exact 2-pass (4 cyc/row), F32R = 1-pass (1 cyc/row).
#   TRANS_DT: dtype of the natural patches tiles + transpose PSUM outs.
#   IDENT_DT: dtype of the identity streamed by the transpose matmul —
#             sets the transpose stream rate (f32 2 / f32r 1.5 / bf16 1
#             cycles per row).
SIM_DT = F32R
TRANS_DT = F32R
IDENT_DT = F32R
LAG_D = 1
SPLIT_LOAD = True
SCATTER_SCALAR = False
NAT_BUFS = 2
PSS_BUFS = 3
PST_BUFS = 2
MAX_WAITS = 1
PT_BUFS = 10


def split_multiwait_ctrl(nc, max_waits=1):
    """Walrus (neuronxcc CoreV3) rejects instructions carrying more than
    one sync wait. Hoist excess waits onto same-engine NOPs emitted just
    before the instruction — program order on the engine's sequencer makes
    this semantically identical (waits are a conjunction)."""
    n_split = 0
    for fn in nc.m.functions:
        for bb in fn.blocks:
            new_list = []
            for inst in bb.instructions:
                si = inst.sync_info
                lim = 1 if isinstance(inst, mybir.InstMatmult) else max_waits
                if si is not None and si.on_wait and len(si.on_wait) > lim:
                    waits = list(si.on_wait)
                    extra, keep = waits[:-lim], waits[-lim:]
                    for i, w in enumerate(extra):
                        d = mybir.InstNoOp(
                            name=f"{inst.name}-ws{i}",
                            engine=inst.engine,
                            ins=[],
                            outs=[],
                            sync_info=mybir.SyncInfo(on_wait=[w], on_update=[]),
                        )
                        nc.register_instruction(d)
                        new_list.append(d)
                    si.on_wait = keep
                    n_split += 1
                new_list.append(inst)
            bb.instructions[:] = new_list
    return n_split


def build_bass():
    nc = bass.Bass(
        trn_type="TRN2",
        target_bir_lowering=False,
        debug=False,
        enable_asserts=False,
    )

    cue_d = nc.dram_tensor("cue", [NS * K, D], F32, kind="ExternalInput").ap()
    pat_d = nc.dram_tensor("patches", [NS * HW, D], F32, kind="ExternalInput").ap()
    drt_d = nc.dram_tensor("drt", [K, 9], F32, kind="ExternalInput").ap()
    dct_d = nc.dram_tensor("dct", [K, 9], F32, kind="ExternalInput").ap()
    wsel_d = nc.dram_tensor("wsel", [K * 9, K], F32, kind="ExternalInput").ap()
    out_d = nc.dram_tensor("out", [NS * K, D], F32, kind="ExternalOutput").ap()

    with tile.TileContext(nc) as tc:
        build_kernel(tc, out_d, cue_d, pat_d, drt_d, dct_d, wsel_d)
    split_multiwait_ctrl(nc, max_waits=MAX_WAITS)
    return nc


def build_kernel(tc, out_d, cue_d, pat_d, drt_d, dct_d, wsel_d):
    nc = tc.nc
    from contextlib import ExitStack

    ctx = ExitStack()
    const = ctx.enter_context(tc.tile_pool(name="const", bufs=1))
    natp = ctx.enter_context(tc.tile_pool(name="nat", bufs=NAT_BUFS))
    ptp = ctx.enter_context(tc.tile_pool(name="pt", bufs=PT_BUFS))
    smallp = ctx.enter_context(tc.tile_pool(name="small", bufs=3))
    gp = ctx.enter_context(tc.tile_pool(name="gat", bufs=3))
    pst = ctx.enter_context(tc.tile_pool(name="ps_t", bufs=PST_BUFS, space="PSUM"))
    pss = ctx.enter_context(tc.tile_pool(name="ps_s", bufs=PSS_BUFS, space="PSUM"))

    # ---- prefetch sample 0's patches ahead of everything ----
    nat0 = natp.tile([P, NHWC, D], TRANS_DT, tag="nat")
    nat0_src = pat_d[:HW, :].rearrange("(c p) d -> p c d", p=P).bitcast(TRANS_DT)
    nc.sync.dma_start(out=nat0[:, : NHWC // 2], in_=nat0_src[:, : NHWC // 2])
    nc.sync.dma_start(out=nat0[:, NHWC // 2 :], in_=nat0_src[:, NHWC // 2 :])

    # ---- constants ----
    if IDENT_DT == F32:
        ident = const.tile([P, P], F32)
        make_identity(nc, ident[:])
    else:
        # walrus rejects memset/affine_select on f32r tiles; build in f32
        # and DVE-copy (a valid f32r "rounding" producer)
        ident_f = const.tile([P, P], F32)
        make_identity(nc, ident_f[:])
        ident = const.tile([P, P], IDENT_DT)
        nc.vector.tensor_copy(out=ident[:], in_=ident_f[:])
    drt = const.tile([K, 9], F32)
    dct = const.tile([K, 9], F32)
    wsel = const.tile([K * 9, K], F32)
    nc.sync.dma_start(out=drt[:], in_=drt_d[:])
    nc.sync.dma_start(out=dct[:], in_=dct_d[:])
    nc.sync.dma_start(out=wsel[:], in_=wsel_d[:])

    # ---- cue -> cueT ----
    # cue_sb [40, 1024] (row = s*5+k); cueT [128, dc, 40] with d = dc*128 + p
    cue_sb = const.tile([NS * K, D], F32)
    nc.scalar.dma_start(out=cue_sb[:], in_=cue_d[:])
    ident_cue = const.tile([NS * K, NS * K], F32)
    make_identity(nc, ident_cue[:])
    cueT = const.tile([P, NDC, NS * K], SIM_DT)
    for dc in range(NDC):
        ps = pst.tile([P, 512], F32, tag="pst")
        nc.tensor.transpose(
            out=ps[:, : NS * K],
            in_=cue_sb[:, dc * P : (dc + 1) * P],
            identity=ident_cue[:],
        )
        nc.vector.tensor_copy(out=cueT[:, dc, :], in_=ps[:, : NS * K])

    # ---- per-sample pipeline ----
    # Software-pipelined across samples: sample s's ROI stage (which sits
    # behind the serial argmax->scatter->gather chain) is issued AFTER
    # sample s+1's transpose+sim stream, so the PE never stalls on it and
    # HAM stays warm.
    def stage_front(s):
        if s == 0:
            nat = nat0
        else:
            nat = natp.tile([P, NHWC, D], TRANS_DT, tag="nat")
            nat_src = pat_d[s * HW : (s + 1) * HW, :].rearrange(
                "(c p) d -> p c d", p=P
            ).bitcast(TRANS_DT)
            nc.sync.dma_start(out=nat[:, : NHWC // 2], in_=nat_src[:, : NHWC // 2])
            nc.sync.dma_start(out=nat[:, NHWC // 2 :], in_=nat_src[:, NHWC // 2 :])

        sim_ps = pss.tile([K, HW], F32, tag="simroi")

        def sim_mms(dc, pt):
            for half in range(2):
                nc.tensor.matmul(
                    out=sim_ps[:, half * 512 : (half + 1) * 512],
                    lhsT=cueT[:, dc, s * K : (s + 1) * K],
                    rhs=pt[:, half * 512 : (half + 1) * 512],
                    start=(dc == 0),
                    stop=(dc == NDC - 1),
                    skip_group_check=True,
                )

        # sim matmuls lag the transposes by LAG_D dc's so the PSUM->SBUF
        # copy latency hides under PE transpose work
        pending = []
        for dc in range(NDC):
            pt = ptp.tile([P, HW], SIM_DT, tag="pt")
            for half in range(2):
                ps = pst.tile([P, 512], TRANS_DT, tag="pst")
                for q in range(4):
                    hwc = half * 4 + q
                    nc.tensor.matmul(
                        out=ps[:, q * P : (q + 1) * P],
                        lhsT=nat[:, hwc, dc * P : (dc + 1) * P],
                        rhs=ident[:],
                        is_transpose=True,
                        skip_group_check=True,
                    )
                dst = pt[:, half * 512 : (half + 1) * 512]
                if half == 0:
                    nc.vector.tensor_copy(out=dst, in_=ps[:])
                else:
                    nc.scalar.copy(out=dst, in_=ps[:])
            pending.append((dc, pt))
            if len(pending) > LAG_D:
                sim_mms(*pending.pop(0))
        for item in pending:
            sim_mms(*item)

        return s, sim_ps

    def stage_mid(s, sim_ps):
        # ---- argmax (issued one sample late: all deps are ready, so these
        # ops never head-of-line-block the DVE/ACT queues) ----
        sim_sb = smallp.tile([K, HW], F32, tag="simsb")
        nc.scalar.copy(out=sim_sb[:], in_=sim_ps[:])
        mx8 = smallp.tile([K, 8], F32, tag="mx8")
        idx8 = smallp.tile([K, 8], U32, tag="idx8")
        nc.vector.max(out=mx8[:], in_=sim_sb[:])
        nc.vector.max_index(out=idx8[:], in_max=mx8[:], in_values=sim_sb[:])

        # ---- index math on GPSIMD (keeps the DVE queue free for the next
        # sample's PSUM->SBUF copies; Pool is otherwise idle) ----
        sc = smallp.tile([K, 16], F32, tag="sc")  # 0 h, 1 w, 2 count
        t9 = smallp.tile([K, 9 * 6], F32, tag="t9")
        hh = t9[:, 0:9]
        ww = t9[:, 9:18]
        hc = t9[:, 18:27]
        wc = t9[:, 27:36]
        valid = t9[:, 36:45]
        gidx_f = t9[:, 45:54]
        # h = idx >> 5, w = idx & 31 on uint32, then cast to f32 (exact)
        hw_u = smallp.tile([K, 2], U32, tag="hwu")
        nc.vector.tensor_scalar(
            out=hw_u[:, 0:1], in0=idx8[:, 0:1], scalar1=5, scalar2=None,
            op0=mybir.AluOpType.logical_shift_right,
        )
        nc.vector.tensor_scalar(
            out=hw_u[:, 1:2], in0=idx8[:, 0:1], scalar1=31, scalar2=None,
            op0=mybir.AluOpType.bitwise_and,
        )
        nc.vector.tensor_copy(out=sc[:, 0:2], in_=hw_u[:])  # uint32 -> f32
        # window rows/cols, clamped; validity
        nc.vector.tensor_scalar(
            out=hh, in0=drt[:], scalar1=sc[:, 0:1], scalar2=None,
            op0=mybir.AluOpType.add,
        )
        nc.vector.tensor_scalar(
            out=ww, in0=dct[:], scalar1=sc[:, 1:2], scalar2=None,
            op0=mybir.AluOpType.add,
        )
        nc.vector.tensor_scalar(
            out=hc, in0=hh, scalar1=0.0, scalar2=float(H - 1),
            op0=mybir.AluOpType.max, op1=mybir.AluOpType.min,
        )
        nc.vector.tensor_scalar(
            out=wc, in0=ww, scalar1=0.0, scalar2=float(W - 1),
            op0=mybir.AluOpType.max, op1=mybir.AluOpType.min,
        )
        nc.vector.tensor_tensor(out=hh, in0=hh, in1=hc, op=mybir.AluOpType.is_equal)
        nc.vector.tensor_tensor(out=ww, in0=ww, in1=wc, op=mybir.AluOpType.is_equal)
        nc.vector.tensor_tensor(out=valid, in0=hh, in1=ww, op=mybir.AluOpType.mult)
        # count = row-span * col-span (no reduction needed):
        #   span = min(x+1, 31) - max(x-1, 0) + 1
        cn = smallp.tile([K, 4], F32, tag="cn")
        nc.vector.tensor_scalar(
            out=cn[:, 0:1], in0=sc[:, 0:1], scalar1=1.0, scalar2=float(H - 1),
            op0=mybir.AluOpType.add, op1=mybir.AluOpType.min,
        )
        nc.vector.tensor_scalar(
            out=cn[:, 1:2], in0=sc[:, 0:1], scalar1=-1.0, scalar2=0.0,
            op0=mybir.AluOpType.add, op1=mybir.AluOpType.max,
        )
        nc.vector.tensor_tensor(
            out=cn[:, 0:1], in0=cn[:, 0:1], in1=cn[:, 1:2],
            op=mybir.AluOpType.subtract,
        )
        nc.vector.tensor_scalar(
            out=cn[:, 2:3], in0=sc[:, 1:2], scalar1=1.0, scalar2=float(W - 1),
            op0=mybir.AluOpType.add, op1=mybir.AluOpType.min,
        )
        nc.vector.tensor_scalar(
            out=cn[:, 3:4], in0=sc[:, 1:2], scalar1=-1.0, scalar2=0.0,
            op0=mybir.AluOpType.add, op1=mybir.AluOpType.max,
        )
        nc.vector.tensor_tensor(
            out=cn[:, 2:3], in0=cn[:, 2:3], in1=cn[:, 3:4],
            op=mybir.AluOpType.subtract,
        )
        nc.vector.tensor_scalar(
            out=cn[:, 0:1], in0=cn[:, 0:1], scalar1=1.0, scalar2=None,
            op0=mybir.AluOpType.add,
        )
        nc.vector.tensor_scalar(
            out=cn[:, 2:3], in0=cn[:, 2:3], scalar1=1.0, scalar2=None,
            op0=mybir.AluOpType.add,
        )
        nc.vector.tensor_tensor(
            out=sc[:, 2:3], in0=cn[:, 0:1], in1=cn[:, 2:3],
            op=mybir.AluOpType.mult,
        )
        # gather row index = s*HW + hc*32 + wc  (clamped -> always in bounds)
        nc.vector.tensor_scalar(
            out=gidx_f, in0=hc, scalar1=float(W), scalar2=float(s * HW),
            op0=mybir.AluOpType.mult, op1=mybir.AluOpType.add,
        )
        nc.vector.tensor_tensor(
            out=gidx_f, in0=gidx_f, in1=wc, op=mybir.AluOpType.add
        )
        vscl = t9[:, 36:45]  # 0/1 selector only; division happens at the end
        gidx_u = smallp.tile([K, 9], U32, tag="gidxu")
        nc.vector.tensor_copy(out=gidx_u[:], in_=gidx_f)

        # ---- spread [5,9] -> [45,1] across partitions (tiny SBUF->SBUF DMA) ----
        gidx45 = smallp.tile([K * 9, 1], U32, tag="gidx45")
        vscl45 = smallp.tile([K * 9, 1], F32, tag="vscl45")
        # scalar-engine HWDGE ring — independent FIFO from the big loads
        sc_eng = nc.scalar if SCATTER_SCALAR else nc.sync
        sc_eng.dma_start(out=gidx45[:], in_=gidx_u[:])
        sc_eng.dma_start(out=vscl45[:], in_=vscl)

        # ---- gather 45 rows from DRAM; weighted-sum via PE ----
        g45 = gp.tile([K * 9, D], F32, tag="g45")
        nc.gpsimd.indirect_dma_start(
            out=g45[:],
            out_offset=None,
            in_=pat_d[:],
            in_offset=bass.IndirectOffsetOnAxis(ap=gidx45[:, :1], axis=0),
        )
        w45 = smallp.tile([K * 9, K], F32, tag="w45")
        nc.vector.tensor_scalar(
            out=w45[:], in0=wsel[:], scalar1=vscl45[:, 0:1], scalar2=None,
            op0=mybir.AluOpType.mult,
        )
        return s, w45, g45, sc

    def stage_roi(s, w45, g45, sc):
        roi_ps = pss.tile([K, D], F32, tag="simroi")
        for half in range(2):
            nc.tensor.matmul(
                out=roi_ps[:, half * 512 : (half + 1) * 512],
                lhsT=w45[:],
                rhs=g45[:, half * 512 : (half + 1) * 512],
                start=True,
                stop=True,
                skip_group_check=True,
            )
        out_sb = smallp.tile([K, D], F32, tag="outsb")
        nc.vector.reciprocal(out=sc[:, 3:4], in_=sc[:, 2:3])
        nc.vector.tensor_scalar(
            out=out_sb[:], in0=roi_ps[:], scalar1=sc[:, 3:4], scalar2=None,
            op0=mybir.AluOpType.mult,
        )
        nc.scalar.dma_start(out=out_d[s * K : (s + 1) * K, :], in_=out_sb[:])

    pend_mid = None
    pend_roi = None
    for s in range(NS):
        fr = stage_front(s)
        new_roi = stage_mid(*pend_mid) if pend_mid is not None else None
        if pend_roi is not None:
            stage_roi(*pend_roi)
        pend_mid = fr
        pend_roi = new_roi
    new_roi = stage_mid(*pend_mid)
    if pend_roi is not None:
        stage_roi(*pend_roi)
    stage_roi(*new_roi)

    ctx.close()


def make_const_inputs():
    r = np.arange(9)
    dr = (r // 3 - 1).astype(np.float32)
    dc = (r % 3 - 1).astype(np.float32)
    drt = np.tile(dr[None, :], (K, 1))
    dct = np.tile(dc[None, :], (K, 1))
    wsel = np.zeros((K * 9, K), np.float32)
    for k in range(K):
        wsel[9 * k : 9 * (k + 1), k] = 1.0
    return drt, dct, wsel


def make_in_maps(cue, patches):
    cue = np.ascontiguousarray(np.asarray(cue, np.float32)).reshape(B, K, D)
    patches = np.ascontiguousarray(np.asarray(patches, np.float32)).reshape(
        B, HW, D
    )
    drt, dct, wsel = make_const_inputs()
    in_maps = []
    for c in range(NCORES):
        in_maps.append(
            {
                "cue": np.ascontiguousarray(
                    cue[c * NS : (c + 1) * NS].reshape(NS * K, D)
                ),
                "patches": np.ascontiguousarray(
                    patches[c * NS : (c + 1) * NS].reshape(NS * HW, D)
                ),
                "drt": drt,
                "dct": dct,
                "wsel": wsel,
            }
        )
    return in_maps


_NC_CACHE = None


def get_nc():
    global _NC_CACHE
    if _NC_CACHE is None:
        _NC_CACHE = build_bass()
    return _NC_CACHE


def run(cue, patches, trace=False):
    from concourse.bass_utils import run_bass_kernel_spmd

    nc = get_nc()
    in_maps = make_in_maps(cue, patches)
    res = run_bass_kernel_spmd(
        nc, in_maps, core_ids=list(range(NCORES)), trace=trace
    )
    outs = [r["out"].reshape(NS, K, D) for r in res.results]
    full = np.concatenate(outs, axis=0)
    return full, res


def kernel(cue, patches):
    full, _ = run(cue, patches, trace=False)
    return full



# revision 53
# speedup vs baseline: 1.1653x; 1.1653x over previous
"""BuddyPool kernel for Trainium2 (Bass/Tile), 8-core data-parallel.

Problem: cue (64,5,1024), patches (64,32,32,1024) ->
  sim = einsum('bkd,bhwd->bkhw'); idx = argmax(sim over hw);
  roi = mean of boundary-clamped 3x3 patch window around idx  -> (64,5,1024)

Sharding: batch across 8 cores, 8 samples/core. Inside one core:
  - stream patches[s] (4 MiB) to SBUF in natural [hw, d] layout; the SP
    queue carries the big loads (plus the tiny spread, issued well before
    the next load's deadline) so the serialized DMA resource never idles
  - PE-transpose 128x128 tiles to patchesT [d, hw]; PSUM->SBUF copies
    spread over DVE/Act/Pool per a static schedule
  - sim matmul: cueT[128,K].T @ patchesT[128,512] accumulated over 8
    d-chunks; hw-halves processed sequentially (half-outer) and
    argmax'd per half on DVE straight out of PSUM
  - argmax combine + window/index math on Pool (f32 only, mod/divide)
  - indirect-DMA gather of 45 rows from DRAM with offsets read directly
    from the [5,9] index tile (no spread on the gather path); the weight
    spread [5,9]->[45,1] runs in parallel on SP
  - roi = W[45,5].T @ G[45,1024] in f32r with W = one-hot(k) * valid/cnt
"""

import sys

if "/opt/trn_rl_repo" not in sys.path:
    sys.path.insert(0, "/opt/trn_rl_repo")

import numpy as np

import concourse.bass as bass
import concourse.tile as tile
from concourse import mybir
from concourse.masks import make_identity

P = 128
B = 64          # full batch
NCORES = 8
NS = B // NCORES  # samples per core
K = 5
D = 1024
H = W = 32
HW = H * W      # 1024
NDC = D // P    # 8 d-chunks
NHWC = HW // P  # 8 hw-chunks
F32 = mybir.dt.float32
F32R = mybir.dt.float32r
U32 = mybir.dt.uint32

SIM_DT = F32R
TRANS_DT = F32R
IDENT_DT = F32R
LAG_D = 3
NCHUNK = 8      # hw-chunks per sample load (must divide NHWC)
DSPLIT = 2      # d-splits per hw-chunk: finer DMA grain so the
                # latency-critical gather/out transfers win the serialized
                # DMA resource at the next free point instead of waiting
                # behind megabyte chunks
NAT_BUFS = 3
PSS_BUFS = 2    # sim PSUM tiles, 2 banks each
PST_BUFS = 3    # transpose PSUM tiles, 1 bank each; roi gets the 8th bank
MAX_WAITS = 1
PT_BUFS = 7

# PSUM->SBUF copy engine per (half, dc): v=DVE a=Act.  Pool does no
# copies (its chain ops precede any copies in queue order, so one late
# chain would stall them).  DVE runs the sample's argmax for ~2.4us at
# the start of each h0, so DVE copy slots sit late in h0.
SCHED = [
    ["a", "a", "a", "a", "v", "v", "a", "v"],
    ["v", "a", "v", "a", "v", "a", "v", "a"],
]

AOP = mybir.AluOpType


def split_multiwait_ctrl(nc, max_waits=1):
    """Walrus (neuronxcc CoreV3) rejects instructions carrying more than
    one sync wait. Hoist excess waits onto same-engine NOPs emitted just
    before the instruction — program order on the engine's sequencer makes
    this semantically identical (waits are a conjunction)."""
    n_split = 0
    for fn in nc.m.functions:
        for bb in fn.blocks:
            new_list = []
            for inst in bb.instructions:
                si = inst.sync_info
                lim = 1 if isinstance(inst, mybir.InstMatmult) else max_waits
                if si is not None and si.on_wait and len(si.on_wait) > lim:
                    waits = list(si.on_wait)
                    extra, keep = waits[:-lim], waits[-lim:]
                    for i, w in enumerate(extra):
                        d = mybir.InstNoOp(
                            name=f"{inst.name}-ws{i}",
                            engine=inst.engine,
                            ins=[],
                            outs=[],
                            sync_info=mybir.SyncInfo(on_wait=[w], on_update=[]),
                        )
                        nc.register_instruction(d)
                        new_list.append(d)
                    si.on_wait = keep
                    n_split += 1
                new_list.append(inst)
            bb.instructions[:] = new_list
    return n_split


def build_bass():
    nc = bass.Bass(
        trn_type="TRN2",
        target_bir_lowering=False,
        debug=False,
        enable_asserts=False,
    )

    cue_d = nc.dram_tensor("cue", [NS * K, D], F32, kind="ExternalInput").ap()
    # one extra all-zero row at index NS*HW: invalid window cells gather
    # it so the ROI sum needs no per-cell weights
    pat_d = nc.dram_tensor(
        "patches", [NS * HW + 1, D], F32, kind="ExternalInput"
    ).ap()
    drt_d = nc.dram_tensor("drt", [K, 9], F32, kind="ExternalInput").ap()
    dct_d = nc.dram_tensor("dct", [K, 9], F32, kind="ExternalInput").ap()
    wsel_d = nc.dram_tensor("wsel", [K * 9, K], F32, kind="ExternalInput").ap()
    out_d = nc.dram_tensor("out", [NS * K, D], F32, kind="ExternalOutput").ap()

    with tile.TileContext(nc) as tc:
        build_kernel(tc, out_d, cue_d, pat_d, drt_d, dct_d, wsel_d)
    split_multiwait_ctrl(nc, max_waits=MAX_WAITS)
    return nc


def build_kernel(tc, out_d, cue_d, pat_d, drt_d, dct_d, wsel_d):
    nc = tc.nc
    from contextlib import ExitStack

    ctx = ExitStack()
    const = ctx.enter_context(tc.tile_pool(name="const", bufs=1))
    natp = ctx.enter_context(tc.tile_pool(name="nat", bufs=NAT_BUFS))
    ptp = ctx.enter_context(tc.tile_pool(name="pt", bufs=PT_BUFS))
    smallp = ctx.enter_context(tc.tile_pool(name="small", bufs=3))
    gp = ctx.enter_context(tc.tile_pool(name="gat", bufs=3))
    pst = ctx.enter_context(tc.tile_pool(name="ps_t", bufs=PST_BUFS, space="PSUM"))
    pss = ctx.enter_context(tc.tile_pool(name="ps_s", bufs=PSS_BUFS, space="PSUM"))
    psr = ctx.enter_context(tc.tile_pool(name="ps_r", bufs=1, space="PSUM"))

    CPC = NHWC // NCHUNK
    pat_r = pat_d.bitcast(F32R)

    DW = D // DSPLIT

    def load_nat(s):
        nat = natp.tile([P, NHWC, D], TRANS_DT, tag="nat")
        src = pat_d[s * HW : (s + 1) * HW, :].rearrange(
            "(c p) d -> p c d", p=P
        ).bitcast(TRANS_DT)
        for ch in range(NCHUNK):
            for dh in range(DSPLIT):
                nc.sync.dma_start(
                    out=nat[:, ch * CPC : (ch + 1) * CPC,
                            dh * DW : (dh + 1) * DW],
                    in_=src[:, ch * CPC : (ch + 1) * CPC,
                            dh * DW : (dh + 1) * DW],
                )
        return nat

    # ---- tiny loads FIRST: the cue gates the whole sim pipeline (cueT
    # transposes head the PE queue), so its HWDGE slot must precede the
    # big patch loads' descriptor stream ----
    cue_sb = const.tile([NS * K, D], F32)
    nc.scalar.dma_start(out=cue_sb[:], in_=cue_d[:])
    drt = const.tile([K, 9], F32)
    dct = const.tile([K, 9], F32)
    wsel = const.tile([K * 9, K], F32)
    nc.scalar.dma_start(out=drt[:], in_=drt_d[:])
    nc.scalar.dma_start(out=dct[:], in_=dct_d[:])
    nc.scalar.dma_start(out=wsel[:], in_=wsel_d[:])
    wselr = const.tile([K * 9, K], F32R)
    nc.vector.tensor_copy(out=wselr[:], in_=wsel[:])

    # ---- prefetch sample 0's patches ----
    nat0 = load_nat(0)

    # ---- constants ----
    ident_f = const.tile([P, P], F32)
    make_identity(nc, ident_f[:])
    if IDENT_DT == F32:
        ident = ident_f
    else:
        ident = const.tile([P, P], IDENT_DT)
        nc.vector.tensor_copy(out=ident[:], in_=ident_f[:])
    # ---- cue -> cueT ----
    ident_cue = const.tile([NS * K, NS * K], F32)
    make_identity(nc, ident_cue[:])
    cueT = const.tile([P, NDC, NS * K], SIM_DT)
    for dc in range(NDC):
        ps = pst.tile([P, 512], F32, tag="pst")
        nc.tensor.transpose(
            out=ps[:, : NS * K],
            in_=cue_sb[:, dc * P : (dc + 1) * P],
            identity=ident_cue[:],
        )
        nc.vector.tensor_copy(out=cueT[:, dc, :], in_=ps[:, : NS * K])

    # ---- per-sample stages ----
    def front_half(s, nat, sim_ps, mx16, idx16, combo, half):
        pending = []

        def sim_mm(dc, pt):
            nc.tensor.matmul(
                out=sim_ps[:, half * 512 : (half + 1) * 512],
                lhsT=cueT[:, dc, s * K : (s + 1) * K],
                rhs=pt[:],
                start=(dc == 0),
                stop=(dc == NDC - 1),
                skip_group_check=True,
            )

        for dc in range(NDC):
            pt = ptp.tile([P, 512], SIM_DT, tag="pt")
            ps = pst.tile([P, 512], TRANS_DT, tag="pst")
            for q in range(4):
                hwc = half * 4 + q
                nc.tensor.matmul(
                    out=ps[:, q * P : (q + 1) * P],
                    lhsT=nat[:, hwc, dc * P : (dc + 1) * P],
                    rhs=ident[:],
                    is_transpose=True,
                    skip_group_check=True,
                )
            eng = SCHED[half][dc]
            if eng == "v":
                nc.vector.tensor_copy(out=pt[:], in_=ps[:])
            elif eng == "a":
                nc.scalar.copy(out=pt[:], in_=ps[:])
            else:
                nc.gpsimd.tensor_copy(out=pt[:], in_=ps[:])
            pending.append((dc, pt))
            if len(pending) > LAG_D:
                sim_mm(*pending.pop(0))
        for item in pending:
            sim_mm(*item)

        # per-half argmax straight out of PSUM (DVE); overlaps the other
        # half's PE work (PST_BUFS=4 gives copies enough slack to absorb
        # the ~1.3us DVE occupancy)
        nc.vector.max(
            out=mx16[:, half * 8 : (half + 1) * 8],
            in_=sim_ps[:, half * 512 : (half + 1) * 512],
        )
        nc.vector.max_index(
            out=idx16[:, half * 8 : (half + 1) * 8],
            in_max=mx16[:, half * 8 : (half + 1) * 8],
            in_values=sim_ps[:, half * 512 : (half + 1) * 512],
        )

    def stage_front(s, nat):
        combo = pss.tile([K, HW], F32, tag="combo")
        sim_ps = combo[:]
        mx16 = smallp.tile([K, 16], F32, tag="mx16")
        idx16 = smallp.tile([K, 16], U32, tag="idx16")
        return s, nat, sim_ps, mx16, idx16, combo

    def mid_a(st):
        """Argmax combine + window/index math, all on Pool; weight spread
        on SP (ahead of the next sample's loads in its queue)."""
        # Pool's ALU has no compare/mod/divide opcodes (walrus rejects
        # them), so everything below is add/sub/mult/min/max only.
        s, nat, sim_ps, mx16, idx16, combo = st
        g = nc.gpsimd
        sc = smallp.tile([K, 8], F32, tag="sc")
        # 0:h 1:w 2:count 3:1/count 4:idx 5:tmp 6:sel
        iff = smallp.tile([K, 2], F32, tag="iff")
        g.tensor_copy(out=iff[:], in_=idx16[:, 0:16:8])  # u32 -> f32
        # first-max: half1 wins only on strictly greater.  f32 max values
        # are O(100), so any non-equal pair differs by >= ULP ~ 1e-5 and
        # (m1-m0)*1e9 clamps cleanly to {0, 1}.
        g.tensor_tensor(out=sc[:, 6:7], in0=mx16[:, 8:9], in1=mx16[:, 0:1],
                        op=AOP.subtract)
        g.tensor_scalar(out=sc[:, 6:7], in0=sc[:, 6:7], scalar1=1e9,
                        scalar2=0.0, op0=AOP.mult, op1=AOP.max)
        g.tensor_scalar(out=sc[:, 6:7], in0=sc[:, 6:7], scalar1=1.0,
                        scalar2=None, op0=AOP.min)
        # idx = i0 + sel * (i1 + 512 - i0)
        g.tensor_scalar(out=sc[:, 5:6], in0=iff[:, 1:2], scalar1=512.0,
                        scalar2=None, op0=AOP.add)
        g.tensor_tensor(out=sc[:, 5:6], in0=sc[:, 5:6], in1=iff[:, 0:1],
                        op=AOP.subtract)
        g.tensor_tensor(out=sc[:, 5:6], in0=sc[:, 5:6], in1=sc[:, 6:7],
                        op=AOP.mult)
        g.tensor_tensor(out=sc[:, 4:5], in0=sc[:, 5:6], in1=iff[:, 0:1],
                        op=AOP.add)
        # h = floor(idx/32) via the round-to-int trick on x=(idx-15.5)/32:
        # |frac(x)| <= 15.5/32 < 0.5 strictly (no RNE ties), and adding
        # 1.5*2^23 puts the sum in [2^23, 2^24) where f32 spacing is
        # exactly 1.0 even for x < 0 (plain 2^23 breaks for idx < 16:
        # sums just below 2^23 quantize to halves)
        MAGIC = 12582912.0  # 1.5 * 2^23
        g.tensor_scalar(out=sc[:, 0:1], in0=sc[:, 4:5], scalar1=-15.5,
                        scalar2=1.0 / W, op0=AOP.add, op1=AOP.mult)
        g.tensor_scalar(out=sc[:, 0:1], in0=sc[:, 0:1], scalar1=MAGIC,
                        scalar2=-MAGIC, op0=AOP.add, op1=AOP.add)
        # w = idx - 32*h
        g.tensor_scalar(out=sc[:, 5:6], in0=sc[:, 0:1], scalar1=-float(W),
                        scalar2=None, op0=AOP.mult)
        g.tensor_tensor(out=sc[:, 1:2], in0=sc[:, 4:5], in1=sc[:, 5:6],
                        op=AOP.add)

        t9 = smallp.tile([K, 9 * 6], F32, tag="t9")
        hw9 = t9[:, 0:18]    # hh | ww
        hh = t9[:, 0:9]
        ww = t9[:, 9:18]
        cl9 = t9[:, 18:36]   # clamped hh | ww
        hc = t9[:, 18:27]
        wc = t9[:, 27:36]
        wch = t9[:, 36:45]   # valid / count -> spread source
        gidx_f = t9[:, 45:54]
        g.tensor_scalar(out=hh, in0=drt[:], scalar1=sc[:, 0:1], scalar2=None,
                        op0=AOP.add)
        g.tensor_scalar(out=ww, in0=dct[:], scalar1=sc[:, 1:2], scalar2=None,
                        op0=AOP.add)
        g.tensor_scalar(out=cl9, in0=hw9, scalar1=0.0, scalar2=float(H - 1),
                        op0=AOP.max, op1=AOP.min)
        # validity: d = raw - clamped is a whole number, 0 iff in range,
        # so valid = max(1 - d^2, 0)
        g.tensor_tensor(out=hw9, in0=hw9, in1=cl9, op=AOP.subtract)
        g.tensor_tensor(out=hw9, in0=hw9, in1=hw9, op=AOP.mult)
        g.tensor_scalar(out=hw9, in0=hw9, scalar1=-1.0, scalar2=1.0,
                        op0=AOP.mult, op1=AOP.add)
        g.tensor_scalar(out=hw9, in0=hw9, scalar1=0.0, scalar2=None,
                        op0=AOP.max)
        g.tensor_tensor(out=wch, in0=hh, in1=ww, op=AOP.mult)
        # gather index: valid cells read their clamped row, invalid cells
        # read the all-zero row at ZIDX, so the ROI matmul can use the
        # CONSTANT one-hot wselr and nothing downstream waits on weights
        ZIDX = float(NS * HW)
        g.tensor_scalar(out=gidx_f, in0=hc, scalar1=float(W),
                        scalar2=float(s * HW) - ZIDX, op0=AOP.mult,
                        op1=AOP.add)
        g.tensor_tensor(out=gidx_f, in0=gidx_f, in1=wc, op=AOP.add)
        g.tensor_tensor(out=gidx_f, in0=gidx_f, in1=wch, op=AOP.mult)
        g.tensor_scalar(out=gidx_f, in0=gidx_f, scalar1=ZIDX, scalar2=None,
                        op0=AOP.add)
        gidx_u = smallp.tile([K, 9], U32, tag="gidxu")
        g.tensor_copy(out=gidx_u[:], in_=gidx_f)  # f32 -> u32
        # spread [5,9] -> [45,1] (SP queue; fine-grained load chunks keep
        # its DMA-queue wait short) -- the indirect gather needs one
        # offset per output partition
        gidx45 = smallp.tile([K * 9, 1], U32, tag="gidx45")
        nc.sync.dma_start(out=gidx45[:], in_=gidx_u[:])
        g45 = gp.tile([K * 9, D], F32R, tag="g45")
        g.indirect_dma_start(
            out=g45[:],
            out_offset=None,
            in_=pat_r[:],
            in_offset=bass.IndirectOffsetOnAxis(ap=gidx45[:, :1], axis=0),
        )
        # count = number of valid cells (4-op tree; Pool has no free-axis
        # reduce); 1/count via the exact interpolating quadratic through
        # count in {4, 6, 9}: c^2/216 - 19c/216 + 19/36.  Off the gather
        # path: it is only consumed by the output-copy's scale.
        s3 = smallp.tile([K, 3], F32, tag="s3")
        g.tensor_tensor(out=s3[:], in0=wch[:, 0:3], in1=wch[:, 3:6],
                        op=AOP.add)
        g.tensor_tensor(out=s3[:], in0=s3[:], in1=wch[:, 6:9], op=AOP.add)
        g.tensor_tensor(out=s3[:, 0:1], in0=s3[:, 0:1], in1=s3[:, 1:2],
                        op=AOP.add)
        g.tensor_tensor(out=sc[:, 2:3], in0=s3[:, 0:1], in1=s3[:, 2:3],
                        op=AOP.add)
        g.tensor_scalar(out=sc[:, 3:4], in0=sc[:, 2:3], scalar1=1.0 / 216.0,
                        scalar2=-19.0 / 216.0, op0=AOP.mult, op1=AOP.add)
        g.tensor_tensor(out=sc[:, 3:4], in0=sc[:, 3:4], in1=sc[:, 2:3],
                        op=AOP.mult)
        g.tensor_scalar(out=sc[:, 3:4], in0=sc[:, 3:4], scalar1=19.0 / 36.0,
                        scalar2=None, op0=AOP.add)
        return s, g45, sc, combo

    def stage_roi(st):
        s, g45, sc, host_combo = st
        # roi uses its own single PSUM bank (halves sequential), fully
        # decoupled from the sim tiles so a late gather never stalls the
        # next sample's sim matmuls via bank reuse
        roi_ps = psr.tile([K, 512], F32, tag="roi")
        out_sb = smallp.tile([K, D], F32, tag="outsb")
        for half in range(2):
            lo, hi = half * 512, (half + 1) * 512
            nc.tensor.matmul(
                out=roi_ps[:],
                lhsT=wselr[:],
                rhs=g45[:, lo:hi],
                start=True,
                stop=True,
                skip_group_check=True,
            )
            # copy+scale(1/count) in one Act activation, then store;
            # half0's copy+dma overlap half1's matmul
            nc.scalar.activation(
                out=out_sb[:, lo:hi],
                in_=roi_ps[:],
                func=mybir.ActivationFunctionType.Copy,
                scale=sc[:, 3:4],
            )
            nc.scalar.dma_start(
                out=out_d[s * K : (s + 1) * K, lo:hi], in_=out_sb[:, lo:hi]
            )

    # software pipeline: front(s) | chain(s-1)
    # pipeline: front(s) | chain(s-1) in flight | roi(s-2) emitted between
    # front(s)'s halves, so by the time the PE reaches the roi matmul its
    # gather has been complete for a full sample and never head-of-line
    # blocks the transpose/sim stream
    nats = {0: nat0}
    pend = None
    pend_roi = None
    for s in range(NS):
        if s + 1 < NS:
            nats[s + 1] = load_nat(s + 1)
        ma = mid_a(pend) if pend is not None else None
        st = stage_front(s, nats.pop(s))
        front_half(*st, half=0)
        front_half(*st, half=1)
        if pend_roi is not None:
            stage_roi(pend_roi)
        pend = st
        pend_roi = ma
    ma = mid_a(pend)
    stage_roi(pend_roi)
    stage_roi(ma)

    ctx.close()


def make_const_inputs():
    r = np.arange(9)
    dr = (r // 3 - 1).astype(np.float32)
    dc = (r % 3 - 1).astype(np.float32)
    drt = np.tile(dr[None, :], (K, 1))
    dct = np.tile(dc[None, :], (K, 1))
    wsel = np.zeros((K * 9, K), np.float32)
    for k in range(K):
        wsel[9 * k : 9 * (k + 1), k] = 1.0
    return drt, dct, wsel


def make_in_maps(cue, patches):
    cue = np.ascontiguousarray(np.asarray(cue, np.float32)).reshape(B, K, D)
    patches = np.ascontiguousarray(np.asarray(patches, np.float32)).reshape(
        B, HW, D
    )
    drt, dct, wsel = make_const_inputs()
    in_maps = []
    for c in range(NCORES):
        in_maps.append(
            {
                "cue": np.ascontiguousarray(
                    cue[c * NS : (c + 1) * NS].reshape(NS * K, D)
                ),
                "patches": np.ascontiguousarray(
                    np.concatenate(
                        [
                            patches[c * NS : (c + 1) * NS].reshape(NS * HW, D),
                            np.zeros((1, D), np.float32),
                        ],
                        axis=0,
                    )
                ),
                "drt": drt,
                "dct": dct,
                "wsel": wsel,
            }
        )
    return in_maps


_NC_CACHE = None


def get_nc():
    global _NC_CACHE
    if _NC_CACHE is None:
        _NC_CACHE = build_bass()
    return _NC_CACHE


def run(cue, patches, trace=False):
    from concourse.bass_utils import run_bass_kernel_spmd

    nc = get_nc()
    in_maps = make_in_maps(cue, patches)
    res = run_bass_kernel_spmd(
        nc, in_maps, core_ids=list(range(NCORES)), trace=trace
    )
    outs = [r["out"].reshape(NS, K, D) for r in res.results]
    full = np.concatenate(outs, axis=0)
    return full, res


def kernel(cue, patches):
    full, _ = run(cue, patches, trace=False)
    return full
